# revision 1
# baseline (speedup 1.0000x reference)
"""Multi-head attention on 8 NeuronCores (Trainium2, Bass/Tile).

Problem: B=2, S=2048, E=1024, H=16, D=64 MHA with int mask, fp32.

Sharding (tensor-parallel hint): core c = 4*b + g handles batch b, head
group g (4 heads = a 256-wide slice of E).  Q/K/V projections, scores,
softmax and attention are head-parallel; Wo is row-sharded so each core
emits a partial [S, E] output projection; the host sums the 4 partials
per batch (the all-reduce) and adds bo.

Device pipeline per core (S=2048, local j = h*64+d in [0,256)):
  proj     : compensated-fp8 DoubleRow matmuls - x and W are split into
             e4m3 hi+lo planes on the host; each 256-contraction pair
             runs 1 main DR (hi*hi, both blocks) + 2 cross DRs
             (w_lo*x_hi + w_hi*x_lo), lo*lo dropped.  0.5 cycles/row.
  qhT, khT : [j, S] fp16 (pair-major [128, pair, S]).
  vh       : [S, j] as [128, s_tile, head, 65] fp16 with a ones column.
  scores   : [ks, q] fp16 matmuls into PSUM; ACT exp (scale=1/8) ->
             fp16; DVE mask multiply (fp16 2x mode).
  ctx      : TRANSPOSED accumulation - stationary = p tile [keys, q],
             moving = vh [keys, d] -> psum [q, d]; 64 rows/call (half
             the rows of the [d, q] form); the denominator accumulates
             separately via 1-row matmuls against the vh ones column.
             Normalize via per-partition reciprocal + tensor_scalar_mul;
             PE-transpose back to [j, q] for the output projection.
  out      : ctxT.T @ WoT on PE, fp16 DMA out (partial, host-reduced).

GPSIMD never touches PSUM (ISA restriction); PSUM memsets use DVE with
a 4-byte view for fp16 tiles.
"""

import os
import sys

sys.path.insert(0, "/opt/trn_rl_repo")

import numpy as np

import concourse.mybir as mybir
import concourse.tile as tile
from concourse import bacc
from concourse import bass_utils

B, S, E, H = 2, 2048, 1024, 16
D = E // H              # 64
G = 4                   # head groups (cores per batch)
HL = H // G             # 4 local heads per core
J = HL * D              # 256 local j width
P = 128
KT = E // P             # 8 k-tiles for projections
ST = S // P             # 16 s-tiles / ks-tiles
NQ = 1024               # q-chunk width for attention
QC = S // NQ            # 2 q chunks
QT = NQ // P            # 8 q-tiles per chunk
MC = 4                  # mask ks-tiles per DMA chunk

F32 = mybir.dt.float32
F16 = mybir.dt.float16
F8 = mybir.dt.float8e4
DR = mybir.MatmulPerfMode.DoubleRow

# fp8 scaling: x*XS and w*WS quantize to e4m3; psum carries XS*WS*x*w and
# the eviction multiplies by 1/(XS*WS).
XS = 4.0
WS = 128.0
INV_SCALE = 1.0 / (XS * WS)

# Exposed for test.py / bench.py.
LAST_RESULTS = None
LAST_NC = None

# Debug ablation flags (timing experiments only; leave empty for correctness).
DBG = {}

def _f16(x: np.ndarray) -> np.ndarray:
    return np.ascontiguousarray(x, dtype=np.float32).astype(np.float16)


def _fp8_pair(a: np.ndarray, scale: float, order: str) -> np.ndarray:
    """[rows, cols] fp32 -> [rows, 2, cols] e4m3 hi/lo split of a*scale.

    order 'hl' stores (hi, lo); 'lh' stores (lo, hi).
    """
    import ml_dtypes

    s = np.ascontiguousarray(a, np.float32) * np.float32(scale)
    hi = s.astype(ml_dtypes.float8_e4m3)
    lo = (s - hi.astype(np.float32)).astype(ml_dtypes.float8_e4m3)
    pair = (hi, lo) if order == "hl" else (lo, hi)
    return np.ascontiguousarray(np.stack(pair, axis=1))


def _build_program(use_bias_qk: bool, use_bias_v: bool):
    nc = bacc.Bacc("TRN2", target_bir_lowering=False, debug=False, num_devices=8)

    xq8 = nc.dram_tensor("xq8", [E, 2, S], F8, kind="ExternalInput")
    xk8 = nc.dram_tensor("xk8", [E, 2, S], F8, kind="ExternalInput")
    xv8 = nc.dram_tensor("xv8", [E, 2, S], F8, kind="ExternalInput")
    maskT = nc.dram_tensor("maskT", [S, S], F16, kind="ExternalInput")
    wq8 = nc.dram_tensor("wq8", [E, 2, J], F8, kind="ExternalInput")
    wk8 = nc.dram_tensor("wk8", [E, 2, J], F8, kind="ExternalInput")
    wv8 = nc.dram_tensor("wv8", [E, 2, J], F8, kind="ExternalInput")
    woT = nc.dram_tensor("woT", [J, E], F16, kind="ExternalInput")
    ident = nc.dram_tensor("ident", [P, P], F16, kind="ExternalInput")
    bq = nc.dram_tensor("bq", [J], F32, kind="ExternalInput")
    bk = nc.dram_tensor("bk", [J], F32, kind="ExternalInput")
    bv = nc.dram_tensor("bv", [J], F32, kind="ExternalInput")
    out = nc.dram_tensor("out", [S, E], F16, kind="ExternalOutput")

    Copy = mybir.ActivationFunctionType.Copy
    Exp = mybir.ActivationFunctionType.Exp

    with tile.TileContext(nc) as tc:
        with (
            tc.tile_pool(name="consts", bufs=1) as consts,
            tc.tile_pool(name="persist", bufs=1) as persist,
            tc.tile_pool(name="xs", bufs=8) as xs,
            tc.tile_pool(name="xv", bufs=1) as xvpool,
            tc.tile_pool(name="maskp", bufs=4) as maskp,
            tc.tile_pool(name="pwork", bufs=8) as pwork,
            tc.tile_pool(name="cnorm", bufs=8) as cnorm,
            tc.tile_pool(name="osb", bufs=6) as osb,
            tc.tile_pool(name="small", bufs=2) as small,
        ):
            # ---- weights / constants ----
            # w tiles: [P, kt, (lo, hi), J] fp8; x tiles: [P, ktpair, (hi, lo), S].
            wq_sb = consts.tile([P, KT, 2, J], F8, tag="wq")
            wk_sb = consts.tile([P, KT, 2, J], F8, tag="wk")
            wv_sb = consts.tile([P, KT, 2, J], F8, tag="wv")
            wo_sb = consts.tile([P, J // P, E], F16, tag="wo")
            id_sb = consts.tile([P, P], F16, tag="id")

            if use_bias_qk:
                bq_sb = consts.tile([P, J // P], F32, tag="bq")
                bk_sb = consts.tile([P, J // P], F32, tag="bk")
                nc.sync.dma_start(bq_sb[:], bq.rearrange("(pr p) -> p pr", p=P))
                nc.sync.dma_start(bk_sb[:], bk.rearrange("(pr p) -> p pr", p=P))
            if use_bias_v:
                bv_row = consts.tile([1, J], F32, tag="bvr")
                nc.sync.dma_start(bv_row[:], bv.rearrange("j -> 1 j"))
                bv_bc = consts.tile([P, J], F32, tag="bvb")
                nc.gpsimd.partition_broadcast(bv_bc[:], bv_row[:])

            # ---- persistent activations ----
            qhT = persist.tile([P, 2, S], F16, tag="qhT")
            khT = persist.tile([P, 2, S], F16, tag="khT")
            vh = persist.tile([P, ST, HL, 65], F16, tag="vh")
            ctxT = persist.tile([P, 2, S], F16, tag="ctxT")

            nc.gpsimd.memset(vh[:, :, :, 64:65], 1.0)
            if DBG.get("no_norm"):
                nc.gpsimd.memset(ctxT[:], 0.0)

            # ---- phase A: projections (compensated fp8 DoubleRow) ----
            # Per 256-contraction pair: 1 main DR (hi*hi for both blocks) +
            # 2 cross DRs (per block: w_lo*x_hi + w_hi*x_lo); lo*lo dropped.
            projacc_cm = tc.tile_pool(name="projacc", bufs=8, space="PSUM")
            projacc = projacc_cm.__enter__()

            halfE = KT // 2 * P
            nc.sync.dma_start(
                wq_sb[:, 0:KT // 2, :, :],
                wq8[0:halfE].rearrange("(kt p) c j -> p kt c j", p=P))
            nc.sync.dma_start(
                wq_sb[:, KT // 2:KT, :, :],
                wq8[halfE:2 * halfE].rearrange("(kt p) c j -> p kt c j", p=P))
            xvt = [xvpool.tile([P, 2, 2, S], F8, tag=f"xv{i}", name=f"xv{i}")
                   for i in range(KT // 2)]

            # q and k -> transposed layout [j, s], pair-major.  DMA order is
            # interleaved so kh tiles arrive before qh compute finishes.
            xq_tiles = [xs.tile([P, 2, 2, S], F8, tag="xt", name=f"xq_t{i}")
                        for i in range(KT // 2)]
            xk_tiles = [xs.tile([P, 2, 2, S], F8, tag="xt", name=f"xk_t{i}")
                        for i in range(KT // 2)]

            def emit_x_dma(xt, x_dram, bp, fine=False):
                if fine:
                    for i in range(2):
                        for c in range(2):
                            nc.sync.dma_start(
                                xt[:, i, c, :],
                                x_dram[(2 * bp + i) * P:
                                       (2 * bp + i + 1) * P, c, :],
                            )
                else:
                    nc.sync.dma_start(
                        xt[:],
                        x_dram[bp * 2 * P:(bp + 1) * 2 * P].rearrange(
                            "(two p) c s -> p two c s", p=P),
                    )

            emit_x_dma(xq_tiles[0], xq8, 0, fine=True)
            emit_x_dma(xq_tiles[1], xq8, 1)
            emit_x_dma(xq_tiles[2], xq8, 2)
            nc.sync.dma_start(wk_sb[:],
                              wk8.rearrange("(kt p) c j -> p kt c j", p=P))
            emit_x_dma(xk_tiles[0], xk8, 0, fine=True)
            emit_x_dma(xq_tiles[3], xq8, 3)
            emit_x_dma(xk_tiles[1], xk8, 1)
            emit_x_dma(xk_tiles[2], xk8, 2)
            emit_x_dma(xk_tiles[3], xk8, 3)

            for w_sb, xtiles, outT, b_sb in (
                (wq_sb, xq_tiles, qhT, "bq"),
                (wk_sb, xk_tiles, khT, "bk"),
            ):
                accs = [projacc.tile([P, 512], F32, tag="pacc", name=f"pacc{i}")
                        for i in range(8)]
                for bp in range(KT // 2):
                    xt = xtiles[bp]
                    for pair in range(2):
                        for n4 in range(4):
                            acc = accs[pair * 4 + n4][:]
                            nsl = slice(n4 * 512, (n4 + 1) * 512)
                            psl = slice(pair * P, (pair + 1) * P)
                            for i in range(2):
                                nc.tensor.matmul(
                                    acc, w_sb[:, 2 * bp + i, :, psl],
                                    xt[:, i, :, nsl],
                                    start=(bp == 0 and i == 0), stop=False,
                                    perf_mode=DR,
                                )
                            nc.tensor.matmul(
                                acc, w_sb[:, 2 * bp:2 * bp + 2, 1, psl],
                                xt[:, :, 0, nsl],
                                start=False,
                                stop=(bp == KT // 2 - 1), perf_mode=DR,
                            )
                for pair in range(2):
                    for n4 in range(4):
                        dst = outT[:, pair, n4 * 512:(n4 + 1) * 512]
                        src = accs[pair * 4 + n4][:]
                        if use_bias_qk:
                            bias = (bq_sb if b_sb == "bq" else bk_sb)[:, pair:pair + 1]
                            nc.scalar.activation(dst, src, Copy, bias=bias,
                                                 scale=INV_SCALE)
                        elif n4 % 2 == 0:
                            nc.vector.tensor_scalar_mul(dst, src, INV_SCALE)
                        else:
                            nc.scalar.activation(dst, src, Copy,
                                                 scale=INV_SCALE)
            # v inputs resident (reused as stationary per s-tile)
            nc.sync.dma_start(wv_sb[:], wv8.rearrange("(kt p) c j -> p kt c j", p=P))
            for bp in range(KT // 2):
                nc.sync.dma_start(
                    xvt[bp][:],
                    xv8[bp * 2 * P:(bp + 1) * 2 * P].rearrange(
                        "(two p) c s -> p two c s", p=P),
                )
            nc.sync.dma_start(id_sb[:], ident[:, :])
            # mask chunk (qc0, ci0) prefetch behind the xv stream.
            pre_mch = maskp.tile([P, MC, NQ], F16, tag="mch", name="mch0_0")
            nc.sync.dma_start(
                pre_mch[:],
                maskT[0:MC * P, 0:NQ].rearrange("(kt p) q -> p kt q", p=P),
            )
            nc.sync.dma_start(wo_sb[:], woT.rearrange("(kt p) e -> p kt e", p=P))
            # v -> natural layout [s, j]; two psum half-passes of 8 s-tiles.
            for sh in range(2):
                vaccs = [projacc.tile([P, J], F32, tag="pacc", name=f"vacc{sh}_{i}")
                         for i in range(8)]
                for bp in range(KT // 2):
                    for si in range(8):
                        st = sh * 8 + si
                        acc = vaccs[si][:]
                        ssl = slice(st * P, (st + 1) * P)
                        nc.tensor.matmul(
                            acc, xvt[bp][:, :, 0, ssl],
                            wv_sb[:, 2 * bp:2 * bp + 2, 1, :],
                            start=(bp == 0), stop=False, perf_mode=DR,
                        )
                        for i in range(2):
                            nc.tensor.matmul(
                                acc, xvt[bp][:, i, :, ssl],
                                wv_sb[:, 2 * bp + i, :, :],
                                start=False,
                                stop=(bp == KT // 2 - 1 and i == 1),
                                perf_mode=DR,
                            )
                for si in range(8):
                    st = sh * 8 + si
                    src3 = vaccs[si][:].rearrange("p (h d) -> p h d", h=HL)
                    dst = vh[:, st, :, 0:64]
                    if use_bias_v:
                        nc.vector.scalar_tensor_tensor(
                            dst, src3, INV_SCALE,
                            bv_bc[:].rearrange("p (h d) -> p h d", h=HL),
                            mybir.AluOpType.mult, mybir.AluOpType.add,
                        )
                    elif si % 2 == 0:
                        nc.vector.tensor_scalar_mul(dst, src3, INV_SCALE)
                    else:
                        nc.scalar.activation(dst, src3, Copy,
                                             scale=INV_SCALE)

            projacc_cm.__exit__(None, None, None)

            # ---- phase B: attention ----
            # PSUM: stps 2x2 banks, cacc 2x1 bank (per-hh), den 1, tp 1 = 8.
            stps_cm = tc.tile_pool(name="stps", bufs=2, space="PSUM")
            stps = stps_cm.__enter__()
            ctxps_cm = tc.tile_pool(name="ctxps", bufs=2, space="PSUM")
            ctxps = ctxps_cm.__enter__()
            denps_cm = tc.tile_pool(name="denps", bufs=1, space="PSUM")
            denps = denps_cm.__enter__()
            tps_cm = tc.tile_pool(name="tps", bufs=1, space="PSUM")
            tps = tps_cm.__enter__()

            den = denps.tile([P, QC, 2, 2, QT], F32, tag="den", name="den")
            nc.vector.memset(den[:], 0.0)
            tp = tps.tile([P, QT, P], F16, tag="tp", name="tp")
            # fp16 memset to PSUM is not ISA-legal; zero via a 4-byte view.
            nc.vector.memset(tp[:].bitcast(F32), 0.0)
            deferred_norm = [None]
            pending = []  # software-pipelined ctx emission (depth 2),
                          # carried across (qc, hp) boundaries

            def emit_ctx(caccs, eqc, ehp, p_t, ks, hh):
                h = 2 * ehp + hh
                last = ks == ST - 1
                for qt in range(QT):
                    stat = p_t[:, qt * P:(qt + 1) * P]
                    if not DBG.get("no_ctx"):
                        nc.tensor.matmul(
                            caccs[hh][:, qt, :], stat,
                            vh[:, ks, h, 0:64],
                            start=False, stop=last,
                            skip_group_check=True,
                        )
                    if not DBG.get("no_den"):
                        nc.tensor.matmul(
                            den[:, eqc, ehp, hh, qt:qt + 1], stat,
                            vh[:, ks, h, 64:65],
                            start=False, stop=last,
                            skip_group_check=True,
                        )

            for qc in range(QC):
                mtiles = {}
                for hp in range(2):          # head pair
                    caccs = []
                    for hh in range(2):
                        cacc = ctxps.tile([P, QT, 64], F32, tag="cacc",
                                          name=f"cacc{qc}_{hp}_{hh}")
                        nc.vector.memset(cacc[:], 0.0)
                        caccs.append(cacc)

                    for ks in range(ST):
                        ci = ks // MC
                        if hp == 0 and ks % MC == 0:
                            if qc == 0 and ci == 0:
                                mtiles[ci] = pre_mch
                            else:
                                mch = maskp.tile([P, MC, NQ], F16, tag="mch",
                                                 name=f"mch{qc}_{ci}")
                                nc.sync.dma_start(
                                    mch[:],
                                    maskT[ks * P:(ks + MC) * P,
                                          qc * NQ:(qc + 1) * NQ].rearrange(
                                        "(kt p) q -> p kt q", p=P),
                                )
                                mtiles[ci] = mch
                        mcur = mtiles[ci]
                        for hh in range(2):  # head within pair -> PE row group
                            st_ = stps.tile([P, NQ], F32, tag="st")
                            for n2 in range(2):
                                nc.tensor.matmul(
                                    st_[:, n2 * 512:(n2 + 1) * 512],
                                    khT[hh * 64:(hh + 1) * 64, hp,
                                        ks * P:(ks + 1) * P],
                                    qhT[hh * 64:(hh + 1) * 64, hp,
                                        qc * NQ + n2 * 512:
                                        qc * NQ + (n2 + 1) * 512],
                                    start=True, stop=True,
                                )
                            if DBG.get("no_exp"):
                                continue
                            p_t = pwork.tile([P, NQ], F16, tag="pt")
                            nc.scalar.activation(p_t[:], st_[:], Exp,
                                                 scale=0.125)
                            if not DBG.get("no_mask"):
                                nc.vector.tensor_mul(p_t[:], p_t[:],
                                                     mcur[:, ks % MC, :])
                            if len(pending) >= DBG.get("pdepth", 3):
                                emit_ctx(*pending.pop(0))
                            pending.append((caccs, qc, hp, p_t, ks, hh))
                            if ks == 2 and hh == 1 and deferred_norm[0]:
                                fn = deferred_norm[0]
                                deferred_norm[0] = None
                                fn()
                    if not DBG.get("carry"):
                        for args in pending:
                            emit_ctx(*args)
                        pending = []

                    # normalize [q, d] by the denominator column, transpose
                    # back into ctxT [j, q] on the PE.  Deferred into the next
                    # (qc, hp) pass's early iterations to keep PE/ACT dense.
                    if DBG.get("no_norm"):
                        continue

                    def norm_block(qc=qc, hp=hp, caccs=caccs):
                        rrs = []
                        for hh in range(2):
                            rr = small.tile([P, QT], F32, tag="rr",
                                            name=f"rr{qc}_{hp}_{hh}")
                            nc.vector.reciprocal(rr[:], den[:, qc, hp, hh, :])
                            rrs.append(rr)
                        cns = []
                        for qt in range(QT):
                            cn = cnorm.tile([P, 2, 64], F16, tag="cn")
                            for hh in range(2):
                                nc.vector.tensor_scalar_mul(
                                    cn[:, hh, :], caccs[hh][:, qt, 0:64],
                                    rrs[hh][:, qt:qt + 1],
                                )
                            cns.append(cn)
                        for qt in range(QT):
                            nc.tensor.matmul(
                                tp[:, qt, :],
                                cns[qt][:].rearrange("p a b -> p (a b)"),
                                id_sb[:], is_transpose=True,
                                start=False, stop=True,
                                skip_group_check=True,
                            )
                            nc.vector.tensor_copy(
                                ctxT[:, hp,
                                     qc * NQ + qt * P:qc * NQ + (qt + 1) * P],
                                tp[:, qt, :],
                            )
                        if not (qc == QC - 1 and hp == 1):
                            nc.vector.memset(tp[:].bitcast(F32), 0.0)

                    if DBG.get("carry") or DBG.get("defer_norm"):
                        deferred_norm[0] = norm_block
                    else:
                        norm_block()
            for args in pending:
                emit_ctx(*args)
            pending = []
            if deferred_norm[0] and not DBG.get("no_norm"):
                deferred_norm[0]()
                deferred_norm[0] = None
            tps_cm.__exit__(None, None, None)
            denps_cm.__exit__(None, None, None)
            ctxps_cm.__exit__(None, None, None)
            stps_cm.__exit__(None, None, None)

            # ---- phase C: output projection (partial) ----
            outps_cm = tc.tile_pool(name="outps", bufs=4, space="PSUM")
            outps = outps_cm.__enter__()
            for st in range(ST):
                ops = [outps.tile([P, 512], F32, tag="ops", name=f"ops{st}_{e}")
                       for e in range(2)]
                for ec in range(2):
                    for kt2 in range(2):
                        nc.tensor.matmul(
                            ops[ec][:],
                            ctxT[:, kt2, st * P:(st + 1) * P],
                            wo_sb[:, kt2, ec * 512:(ec + 1) * 512],
                            start=(kt2 == 0), stop=(kt2 == 1),
                        )
                o_sb = osb.tile([P, E], F16, tag="o")
                nc.scalar.activation(o_sb[:, 0:512], ops[0][:], Copy)
                nc.vector.tensor_copy(o_sb[:, 512:1024], ops[1][:])
                nc.sync.dma_start(out[st * P:(st + 1) * P, :], o_sb[:])
            outps_cm.__exit__(None, None, None)

    nc.compile()
    return nc


def kernel(q, k, v, mask, Wq, bq, Wk, bk, Wv, bv, Wo, bo):
    global LAST_RESULTS
    q = np.asarray(q, np.float32)
    k = np.asarray(k, np.float32)
    v = np.asarray(v, np.float32)
    mask = np.asarray(mask)
    Wq = np.asarray(Wq, np.float32)
    Wk = np.asarray(Wk, np.float32)
    Wv = np.asarray(Wv, np.float32)
    Wo = np.asarray(Wo, np.float32)
    bq = np.asarray(bq, np.float32)
    bk = np.asarray(bk, np.float32)
    bv = np.asarray(bv, np.float32)
    bo = np.asarray(bo, np.float32)

    use_bias_qk = bool(np.any(bq) or np.any(bk))
    use_bias_v = bool(np.any(bv))

    global LAST_NC
    nc = _build_program(use_bias_qk, use_bias_v)
    LAST_NC = nc

    xT = {}
    for b in range(B):
        xT[("q", b)] = _fp8_pair(q[b].T, XS, "hl")
        xT[("k", b)] = _fp8_pair(k[b].T, XS, "hl")
        xT[("v", b)] = _fp8_pair(v[b].T, XS, "hl")
        xT[("m", b)] = _f16(mask[b, 0].T.astype(np.float32))

    eye = _f16(np.eye(P, dtype=np.float32))

    in_maps = []
    for c in range(8):
        b, g = divmod(c, G)
        js = slice(g * J, (g + 1) * J)
        in_maps.append({
            "xq8": xT[("q", b)],
            "xk8": xT[("k", b)],
            "xv8": xT[("v", b)],
            "maskT": xT[("m", b)],
            "wq8": _fp8_pair(Wq[js, :].T, WS, "lh"),
            "wk8": _fp8_pair(Wk[js, :].T, WS, "lh"),
            "wv8": _fp8_pair(Wv[js, :].T, WS, "lh"),
            "woT": _f16(Wo[:, js].T),
            "ident": eye,
            "bq": np.ascontiguousarray(bq[js]),
            "bk": np.ascontiguousarray(bk[js]),
            "bv": np.ascontiguousarray(bv[js]),
        })

    os.environ["BASS_NEVER_TRACE"] = "1"
    res = bass_utils.run_bass_kernel_spmd(
        nc, in_maps, core_ids=list(range(8)), trace=False,
    )
    LAST_RESULTS = res

    full = np.zeros((B, S, E), np.float32)
    for c in range(8):
        b = c // G
        full[b] += res.results[c]["out"].astype(np.float32)
    full += bo[None, None, :]
    return full

